# revision 31
# baseline (speedup 1.0000x reference)
"""Causal self-attention (B=4, T=2048, C=2048, H=16) on 8 trn2 NeuronCores.

Sharding: tensor-parallel over heads - 2 heads per core. Every core gets the
full (pre-transposed) activation xT, its 2 heads' slice of Wqkv columns and
Wproj rows, computes a full [B*T, C] partial output (fp16), and the host sums
the 8 partials (the "all-reduce after output projection" done host-side).

Per-core dataflow (all matmuls fp16 on PE), v2 schedule:
  * Q^T,K^T [d,t] via W-stationary matmuls; V [t,d] computed DIRECTLY via
    x^T-stationary matmuls (no PE transposes).
  * S = K^T-block.T @ Q^T chunks (PSUM f32) -> exp (ACT) -> causal zeroing of
    diagonal blocks on GPSIMD (affine_select, off the PE) -> P (fp16).
  * softmax denominator chased on DVE in fp16; partition-reduced by one
    ones-column matmul; reciprocal broadcast by one ones-row matmul.
  * The whole emission is a single interleaved stream: QKV matmuls of the
    NEXT chunk and output-projection matmuls of the PREVIOUS (b,qg) group are
    woven between attention S/PV blocks, so the in-order PE queue always has
    exp-independent work while ACT streams the exponentials.
  * PSUM budget (8 banks): S-blocks ring 3, QKV accum 1, PV accum ring 2,
    shared transient ring 2 (proj out / V accum / den row / recip bcast).
"""
import numpy as np

B, T, C = 4, 2048, 2048
H, HD = 16, 128
N_CORES = 8
HPC = H // N_CORES          # heads per core = 2
SCALE = float(1.0 / np.sqrt(HD))

_CACHE = {}


def _build_nc():
    import concourse.bass as bass
    from concourse import bacc
    import concourse.tile as tile
    import concourse.mybir as mybir
    from contextlib import ExitStack

    f32 = mybir.dt.float32
    f32r = mybir.dt.float32r
    f16 = mybir.dt.float16
    Exp = mybir.ActivationFunctionType.Exp
    IsGe = mybir.AluOpType.is_ge

    nc = bacc.Bacc("TRN2", target_bir_lowering=False, debug=False,
                   enable_asserts=True, num_devices=N_CORES)

    # Inputs (per-core shards prepared on host)
    xT = nc.dram_tensor("xt", [C, B * T], f16, kind="ExternalInput").ap()
    wqkv = nc.dram_tensor("wqkv", [C, 6 * HD], f16, kind="ExternalInput").ap()
    wproj = nc.dram_tensor("wproj", [HPC * HD, C], f16, kind="ExternalInput").ap()
    out = nc.dram_tensor("out", [B * T, C], f16, kind="ExternalOutput").ap()

    # DRAM views
    # j-major weight view: one DMA per qkv column-block j (j: q0,q1,k0,k1,v0,v1)
    wqkv_v = wqkv.rearrange("(cc p) (j d) -> p j cc d", p=128, d=HD)  # [128,6,16,128]
    wproj_v = wproj.rearrange("(jh p) c -> p jh c", p=128)            # [128,2,2048]
    xv = xT.rearrange("(cc p) t -> p cc t", p=128)                    # [128,16,8192]

    NCC = C // 128        # 16 contraction chunks
    SEQ = [(b, qg) for b in range(B) for qg in range(4)]

    with tile.TileContext(nc) as tc, ExitStack() as ctx:
        const = ctx.enter_context(tc.tile_pool(name="const", bufs=1))
        wpool = ctx.enter_context(tc.tile_pool(name="w", bufs=1))
        xtp = ctx.enter_context(tc.tile_pool(name="xt", bufs=2))
        qkvp = ctx.enter_context(tc.tile_pool(name="qkv", bufs=2))
        ptp = ctx.enter_context(tc.tile_pool(name="pt", bufs=2))
        dnp = ctx.enter_context(tc.tile_pool(name="dn", bufs=3))
        rp = ctx.enter_context(tc.tile_pool(name="r", bufs=2))
        ytp = ctx.enter_context(tc.tile_pool(name="yt", bufs=3))
        op = ctx.enter_context(tc.tile_pool(name="o", bufs=4))
        ps = ctx.enter_context(tc.tile_pool(name="ps", bufs=1, space="PSUM"))

        ones_col = const.tile([128, 1], f16)
        nc.vector.memset(ones_col, 1.0)
        ones_row = const.tile([1, 128], f16)
        nc.vector.memset(ones_row, 1.0)
        dmy = const.tile([128, 512], f16)
        nc.vector.memset(dmy, 0.0)

        w_sb = wpool.tile([128, 6, NCC, HD], f16)
        wp_sb = wpool.tile([128, 2, C], f16)

        # ---------- chunk (QKV) machinery ----------
        xt_pend = {}
        sets = {}

        def dma_xt(c):
            b, qg = c
            t0 = b * T + qg * 512
            xt_t = xtp.tile([128, NCC, 512], f16, tag="xt", name=f"xt{b}{qg}")
            for g in range(4):
                nc.sync.dma_start(
                    xt_t[:, 4 * g:4 * g + 4, :], xv[:, 4 * g:4 * g + 4, t0:t0 + 512])
            xt_pend[c] = xt_t

        def alloc_set(b):
            qt = qkvp.tile([128, HPC, T], f16, tag="qt", name=f"qt{b}")
            kt = qkvp.tile([128, HPC, T], f16, tag="kt", name=f"kt{b}")
            v = qkvp.tile([128, T // 128, HPC * HD], f16, tag="v", name=f"v{b}")
            sets[b] = (qt, kt, v)

        def emit_qk_lump(c, j, kick):
            """16 W-stationary matmuls: one of q_h0/q_h1/k_h0/k_h1 for chunk c."""
            b, qg = c
            if b not in sets:
                alloc_set(b)
            qt, kt, v = sets[b]
            xt_t = xt_pend[c]
            qk_ps = ps.tile([128, 512], f32, tag="qk", bufs=1)
            for cc in range(NCC):
                nc.tensor.matmul(qk_ps, w_sb[:, j, cc, :], xt_t[:, cc, :],
                                 start=(cc == 0), stop=(cc == NCC - 1))
                if cc % 4 == 3:
                    kick()
            dst = (qt, qt, kt, kt)[j]
            nc.vector.tensor_copy(dst[:, j % 2, qg * 512:(qg + 1) * 512], qk_ps)

        def emit_v_lump(c, tb, kick, last=False):
            """16 x^T-stationary matmuls: V[t-block, 2*HD] for chunk c, direct
            [t, d] layout - no transposes."""
            b, qg = c
            if b not in sets:
                alloc_set(b)
            qt, kt, v = sets[b]
            xt_t = xt_pend[c]
            v_ps = ps.tile([128, 2 * HD], f32, tag="ov", bufs=2)
            for cc in range(NCC):
                nc.tensor.matmul(
                    v_ps, xt_t[:, cc, tb * 128:(tb + 1) * 128],
                    w_sb[:, 4:6, cc, :],
                    start=(cc == 0), stop=(cc == NCC - 1))
                if cc % 4 == 3:
                    kick()
            nc.vector.tensor_copy(v[:, qg * 4 + tb, :], v_ps)

        # ---------- attention pipeline (software-pipelined, depth 2) ----------
        PIPE = []

        def pipe_flush():
            kb, qs, st, u = PIPE.pop(0)
            pt, den = u["pt"], u["den"]
            nc.scalar.activation(
                pt[:, kb, qs:512], st[:, qs:512], Exp, scale=SCALE)
            if kb - 4 * u["qg"] >= 0:
                # causal zeroing of the upper triangle of the diagonal
                # 128x128 sub-block - on GPSIMD, off the PE/ACT/DVE hot paths
                nc.gpsimd.affine_select(
                    out=pt[:, kb, qs:qs + 128], in_=pt[:, kb, qs:qs + 128],
                    compare_op=IsGe, fill=0.0,
                    base=0, pattern=[[1, 128]], channel_multiplier=-1)
            if kb == 0:
                nc.vector.tensor_copy(den, pt[:, 0, :])
            else:
                nc.vector.tensor_add(
                    den[:, qs:512], den[:, qs:512], pt[:, kb, qs:512])
            nc.tensor.matmul(
                u["yt_ps"][:, qs:512], u["v_ap"][:, kb, :],
                pt[:, kb, qs:512],
                start=(kb == 0), stop=(kb == u["nkb"] - 1))

        def kick():
            if PIPE:
                pipe_flush()

        def pipe_push(e):
            PIPE.append(e)
            while len(PIPE) > 2:
                pipe_flush()

        def make_unit(b, qg, h):
            qt, kt, v = sets[b]
            return {
                "b": b, "qg": qg, "h": h, "nkb": 4 * qg + 4,
                "pt": ptp.tile([128, T // 128, 512], f16, tag="pt",
                               name=f"pt{b}{qg}{h}"),
                "den": dnp.tile([128, 512], f16, tag="den", name=f"dn{b}{qg}{h}"),
                "yt_ps": ps.tile([128, 512], f32, tag="yt", bufs=2,
                                 name=f"ytps{b}{qg}{h}"),
                "v_ap": v[:, :, h * HD:(h + 1) * HD],
            }

        def emit_block(u, kb):
            b, qg, h = u["b"], u["qg"], u["h"]
            qt, kt, v = sets[b]
            kk = kb - 4 * qg
            qs = max(0, kk) * 128
            st = ps.tile([128, 512], f32, tag="st", bufs=3)
            nc.tensor.matmul(
                st[:, qs:512], kt[:, h, kb * 128:(kb + 1) * 128],
                qt[:, h, qg * 512 + qs:(qg + 1) * 512],
                start=True, stop=True)
            pipe_push((kb, qs, st, u))

        # ---------- softmax epilogue (trail-1) ----------
        def emit_epiA(u):
            den_row = ps.tile([1, 512], f32, tag="ov", bufs=2,
                              name=f"dr{u['b']}{u['qg']}{u['h']}")
            nc.tensor.matmul(den_row, ones_col, u["den"], start=True, stop=True)
            rec = rp.tile([1, 512], f32, tag="rec")
            nc.vector.reciprocal_approx_fast(rec, den_row[0:1, :])
            rec16 = rp.tile([1, 512], f16, tag="rec16")
            nc.scalar.copy(rec16, rec)
            u["rec16"] = rec16

        def emit_epiB(u, yt):
            r_ps = ps.tile([128, 512], f32, tag="ov", bufs=2,
                           name=f"rps{u['b']}{u['qg']}{u['h']}")
            nc.tensor.matmul(r_ps, ones_row, u["rec16"], start=True, stop=True)
            r_sb = rp.tile([128, 512], f32, tag="rsb")
            nc.scalar.copy(r_sb, r_ps)
            nc.vector.tensor_mul(yt[:, u["h"], :], u["yt_ps"], r_sb)

        # ---------- output projection ----------
        osb_pend = {}

        def emit_proj_pair(b, qg, yt, tt, co, single_dma, act_frac=2):
            """act_frac=2: alternate evac DVE/ACT 1:1. act_frac=3: 1 in 3 on
            ACT (for the exp-saturated last step)."""
            if single_dma:
                # drain: the attention st/ov rings are idle by now - borrow
                # both (5 banks) so the final proj pairs are never gated on
                # PSUM-evacuation latency
                if (tt * 4 + co) % 2 == 0:
                    o_ps = ps.tile([128, 512], f32, tag="st", bufs=3,
                                   name=f"ops{b}{qg}{tt}{co}")
                else:
                    o_ps = ps.tile([128, 512], f32, tag="ov", bufs=2,
                                   name=f"ops{b}{qg}{tt}{co}")
            else:
                o_ps = ps.tile([128, 512], f32, tag="ov", bufs=2,
                               name=f"ops{b}{qg}{tt}{co}")
            for jh in range(HPC):
                nc.tensor.matmul(
                    o_ps, yt[:, jh, tt * 128:(tt + 1) * 128],
                    wp_sb[:, jh, co * 512:(co + 1) * 512],
                    start=(jh == 0), stop=(jh == HPC - 1))
            r0 = b * T + qg * 512 + tt * 128
            on_act = (tt * 4 + co) % act_frac == 0
            if single_dma:
                o_sb = op.tile([128, 512], f16, tag="osb1", bufs=4,
                               name=f"os{b}{qg}{tt}{co}")
                if not on_act:
                    nc.vector.tensor_copy(o_sb, o_ps)
                else:
                    nc.scalar.copy(o_sb, o_ps)
                nc.sync.dma_start(
                    out[r0:r0 + 128, co * 512:(co + 1) * 512], o_sb)
                return
            if co % 2 == 0:
                osb_pend[(b, qg)] = op.tile(
                    [128, 1024], f16, tag="osb", bufs=4, name=f"os{b}{qg}{tt}{co}")
            o_sb = osb_pend[(b, qg)]
            dst = o_sb[:, (co % 2) * 512:(co % 2 + 1) * 512]
            # alternate PSUM evacuation between DVE and ACT
            if not on_act:
                nc.vector.tensor_copy(dst, o_ps)
            else:
                nc.scalar.copy(dst, o_ps)
            if co % 2 == 1:
                c2 = co // 2
                nc.sync.dma_start(
                    out[r0:r0 + 128, c2 * 1024:(c2 + 1) * 1024], o_sb)

        # ---------- per-step interleaved emission ----------
        def emit_step(prev, idx, b, qg):
            nkb = 4 * qg + 4
            n2 = 2 * nkb
            ev = []
            seq_n = [0]

            def at(pos, fn):
                seq_n[0] += 1
                ev.append((pos, seq_n[0], fn))

            step_state = {"units": {}, "yt": None}

            def block_fn(h, kb):
                def f():
                    u = step_state["units"].get(h)
                    if u is None:
                        u = step_state["units"][h] = make_unit(b, qg, h)
                        if h == 0:
                            step_state["yt"] = ytp.tile(
                                [128, HPC, 512], f16, tag="yt",
                                name=f"yt{b}{qg}")
                    emit_block(u, kb)
                return f

            for h in range(2):
                for kb in range(nkb):
                    at(h * nkb + kb, block_fn(h, kb))

            if idx + 2 < len(SEQ):
                at(-1.0, (lambda c: lambda: dma_xt(c))(SEQ[idx + 2]))

            if idx + 1 < len(SEQ):
                nxt = SEQ[idx + 1]
                order = [("qk", 0), ("v", 0), ("qk", 2), ("v", 1),
                         ("qk", 1), ("v", 2), ("qk", 3), ("v", 3)]
                if idx == 14:
                    # defer most of chunk (3,3)'s QKV into the final step,
                    # which otherwise has no exp-independent PE filler and
                    # goes engine-bound (and lets HAM re-throttle the PE)
                    order = [("qk", 0), ("qk", 2)]
                for i, (kind, j) in enumerate(order):
                    if kind == "qk":
                        fn = (lambda jj: lambda: emit_qk_lump(nxt, jj, kick))(j)
                    else:
                        fn = (lambda tb: lambda: emit_v_lump(
                            nxt, tb, kick, last=(tb == 3)))(j)
                    at((i + 0.45) * n2 / len(order), fn)
            if idx == 15:
                cur = SEQ[15]
                deferred = [("v", 0, 0.7), ("v", 1, 3.4), ("qk", 1, 6.1),
                            ("v", 2, 9.0), ("v", 3, 12.0), ("qk", 3, 17.5)]
                for kind, j, pos in deferred:
                    if kind == "qk":
                        fn = (lambda jj: lambda: emit_qk_lump(cur, jj, kick))(j)
                    else:
                        fn = (lambda tb: lambda: emit_v_lump(
                            cur, tb, kick))(j)
                    at(pos, fn)

            if prev is not None:
                pu, pyt = prev["h1"], prev["yt"]
                pb, pqg = prev["bqg"]
                at(2.4, (lambda u: lambda: emit_epiA(u))(pu))
                at(4.4, (lambda u, y: lambda: emit_epiB(u, y))(pu, pyt))
                act_frac = 3 if idx == 15 else 2
                span = max(n2 - 6, 2)
                for i in range(16):
                    tt, co = divmod(i, 4)
                    at(5.5 + i * span / 16.0,
                       (lambda t_, c_, a_: lambda: emit_proj_pair(
                           pb, pqg, pyt, t_, c_, False, a_))(tt, co, act_frac))

            at(nkb + 2.4, lambda: emit_epiA(step_state["units"][0]))
            at(nkb + 4.4, lambda: emit_epiB(step_state["units"][0],
                                            step_state["yt"]))

            ev.sort(key=lambda e: (e[0], e[1]))
            for _, _, fn in ev:
                fn()
            return {"h1": step_state["units"][1], "yt": step_state["yt"],
                    "bqg": (b, qg), "prev": prev}

        # ---------- prologue: weights + first chunk ----------
        alloc_set(0)
        # warm-up matmuls: keep the PE busy >4us while the first DMAs land
        # so the HAM clock-gate reaches 8/8 before the real matmul stream
        wu_ps = ps.tile([1, 512], f32, tag="ov", bufs=2, name="wups")

        def warm(n=1):
            for _ in range(n):
                nc.tensor.matmul(wu_ps, ones_col, dmy, start=True, stop=True)

        warm(16)
        # startup DMAs interleaved in first-lump consumption order: 4-cc
        # groups of w_j0 / xt(0,0) / w_j2 so the first matmuls start after
        # ~0.6MB, not after the full 5MB
        xt00 = xtp.tile([128, NCC, 512], f16, tag="xt", name="xt00")
        xt_pend[(0, 0)] = xt00
        for g in range(4):
            nc.sync.dma_start(w_sb[:, 0, 4 * g:4 * g + 4, :],
                              wqkv_v[:, 0, 4 * g:4 * g + 4, :])
            nc.sync.dma_start(xt00[:, 4 * g:4 * g + 4, :],
                              xv[:, 4 * g:4 * g + 4, 0:512])
            nc.sync.dma_start(w_sb[:, 2, 4 * g:4 * g + 4, :],
                              wqkv_v[:, 2, 4 * g:4 * g + 4, :])
        nc.sync.dma_start(w_sb[:, 4], wqkv_v[:, 4])
        nc.sync.dma_start(w_sb[:, 5], wqkv_v[:, 5])
        nop = lambda: None
        # dummy-matmul kicks fill the DMA-starved stretches of the prologue
        # so the HAM clock-gate does not oscillate back to 4/8
        emit_qk_lump((0, 0), 0, warm)
        nc.sync.dma_start(w_sb[:, 1], wqkv_v[:, 1])
        nc.sync.dma_start(w_sb[:, 3], wqkv_v[:, 3])
        emit_qk_lump((0, 0), 2, warm)
        for tb in range(4):
            emit_v_lump((0, 0), tb, warm if tb < 2 else nop, last=(tb == 3))
        emit_qk_lump((0, 0), 1, nop)
        emit_qk_lump((0, 0), 3, nop)
        dma_xt((0, 1))
        nc.sync.dma_start(wp_sb, wproj_v)

        # ---------- main loop ----------
        prev = None
        for idx, (b, qg) in enumerate(SEQ):
            prev = emit_step(prev, idx, b, qg)

        # ---------- drain ----------
        # dummy matmuls keep the PE active through the serial softmax
        # epilogue chain so HAM stays at 8/8 for the final proj matmuls
        wu_d = ps.tile([1, 512], f32, tag="qk", bufs=1, name="wud")

        def warm_d(n):
            for _ in range(n):
                nc.tensor.matmul(wu_d, ones_col, dmy, start=True, stop=True)

        while PIPE:
            pipe_flush()
            warm_d(3)
        emit_epiA(prev["h1"])
        warm_d(10)
        emit_epiB(prev["h1"], prev["yt"])
        warm_d(5)
        for i in range(16):
            tt, co = divmod(i, 4)
            emit_proj_pair(3, 3, prev["yt"], tt, co, True)

    nc.compile()
    return nc


def _get_nc():
    if "nc" not in _CACHE:
        _CACHE["nc"] = _build_nc()
    return _CACHE["nc"]


def _make_in_maps(x2d, Wqkv, Wproj):
    xT = np.ascontiguousarray(x2d.T).astype(np.float16)  # [C, B*T]
    in_maps = []
    for c in range(N_CORES):
        h0 = c * HPC
        cols = []
        for part in range(3):  # q, k, v blocks of Wqkv columns
            for h in range(HPC):
                j0 = part * C + (h0 + h) * HD
                cols.append(Wqkv[:, j0:j0 + HD])
        wq = np.ascontiguousarray(np.concatenate(cols, axis=1)).astype(np.float16)
        wp = np.ascontiguousarray(
            Wproj[h0 * HD:(h0 + HPC) * HD, :]).astype(np.float16)
        in_maps.append({"xt": xT, "wqkv": wq, "wproj": wp})
    return in_maps


def run_shards(in_maps, trace=False):
    from concourse.bass_utils import run_bass_kernel_spmd
    nc = _get_nc()
    last_err = None
    for _attempt in range(3):
        try:
            return run_bass_kernel_spmd(
                nc, in_maps, core_ids=list(range(N_CORES)), trace=trace)
        except Exception as e:  # transient NRT device errors — retry
            last_err = e
            if "UNAVAILABLE" not in str(e) and "UNRECOVERABLE" not in str(e):
                raise
    raise last_err


def kernel(x, Wqkv, Wproj):
    x = np.asarray(x, dtype=np.float32)
    Wqkv = np.asarray(Wqkv, dtype=np.float32)
    Wproj = np.asarray(Wproj, dtype=np.float32)
    x2d = np.ascontiguousarray(x.reshape(B * T, C))

    in_maps = _make_in_maps(x2d, Wqkv, Wproj)
    res = run_shards(in_maps)

    acc = res.results[0]["out"].astype(np.float32)
    for c in range(1, N_CORES):
        acc += res.results[c]["out"].astype(np.float32)
    return acc.reshape(B, T, C)


# revision 33
# speedup vs baseline: 1.0026x; 1.0026x over previous
"""Causal self-attention (B=4, T=2048, C=2048, H=16) on 8 trn2 NeuronCores.

Sharding: tensor-parallel over heads - 2 heads per core. Every core gets the
full (pre-transposed) activation xT, its 2 heads' slice of Wqkv columns and
Wproj rows, computes a full [B*T, C] partial output (fp16), and the host sums
the 8 partials (the "all-reduce after output projection" done host-side).

Per-core dataflow (all matmuls fp16 on PE), v2 schedule:
  * Q^T,K^T [d,t] via W-stationary matmuls; V [t,d] computed DIRECTLY via
    x^T-stationary matmuls (no PE transposes).
  * S = K^T-block.T @ Q^T chunks (PSUM f32) -> exp (ACT) -> causal zeroing of
    diagonal blocks on GPSIMD (affine_select, off the PE) -> P (fp16).
  * softmax denominator chased on DVE in fp16; partition-reduced by one
    ones-column matmul; reciprocal broadcast by one ones-row matmul.
  * The whole emission is a single interleaved stream: QKV matmuls of the
    NEXT chunk and output-projection matmuls of the PREVIOUS (b,qg) group are
    woven between attention S/PV blocks, so the in-order PE queue always has
    exp-independent work while ACT streams the exponentials.
  * PSUM budget (8 banks): S-blocks ring 3, QKV accum 1, PV accum ring 2,
    shared transient ring 2 (proj out / V accum / den row / recip bcast).
"""
import numpy as np

B, T, C = 4, 2048, 2048
H, HD = 16, 128
N_CORES = 8
HPC = H // N_CORES          # heads per core = 2
SCALE = float(1.0 / np.sqrt(HD))

_CACHE = {}


def _build_nc():
    import concourse.bass as bass
    from concourse import bacc
    import concourse.tile as tile
    import concourse.mybir as mybir
    from contextlib import ExitStack

    f32 = mybir.dt.float32
    f32r = mybir.dt.float32r
    f16 = mybir.dt.float16
    Exp = mybir.ActivationFunctionType.Exp
    IsGe = mybir.AluOpType.is_ge

    nc = bacc.Bacc("TRN2", target_bir_lowering=False, debug=False,
                   enable_asserts=True, num_devices=N_CORES)

    # Inputs (per-core shards prepared on host)
    xT = nc.dram_tensor("xt", [C, B * T], f16, kind="ExternalInput").ap()
    wqkv = nc.dram_tensor("wqkv", [C, 6 * HD], f16, kind="ExternalInput").ap()
    wproj = nc.dram_tensor("wproj", [HPC * HD, C], f16, kind="ExternalInput").ap()
    out = nc.dram_tensor("out", [B * T, C], f16, kind="ExternalOutput").ap()

    # DRAM views
    # j-major weight view: one DMA per qkv column-block j (j: q0,q1,k0,k1,v0,v1)
    wqkv_v = wqkv.rearrange("(cc p) (j d) -> p j cc d", p=128, d=HD)  # [128,6,16,128]
    wproj_v = wproj.rearrange("(jh p) c -> p jh c", p=128)            # [128,2,2048]
    xv = xT.rearrange("(cc p) t -> p cc t", p=128)                    # [128,16,8192]

    NCC = C // 128        # 16 contraction chunks
    SEQ = [(b, qg) for b in range(B) for qg in range(4)]

    with tile.TileContext(nc) as tc, ExitStack() as ctx:
        const = ctx.enter_context(tc.tile_pool(name="const", bufs=1))
        wpool = ctx.enter_context(tc.tile_pool(name="w", bufs=1))
        xtp = ctx.enter_context(tc.tile_pool(name="xt", bufs=2))
        qkvp = ctx.enter_context(tc.tile_pool(name="qkv", bufs=2))
        ptp = ctx.enter_context(tc.tile_pool(name="pt", bufs=2))
        dnp = ctx.enter_context(tc.tile_pool(name="dn", bufs=3))
        rp = ctx.enter_context(tc.tile_pool(name="r", bufs=2))
        ytp = ctx.enter_context(tc.tile_pool(name="yt", bufs=3))
        op = ctx.enter_context(tc.tile_pool(name="o", bufs=4))
        ps = ctx.enter_context(tc.tile_pool(name="ps", bufs=1, space="PSUM"))

        ones_col = const.tile([128, 1], f16)
        nc.vector.memset(ones_col, 1.0)
        ones_row = const.tile([1, 128], f16)
        nc.vector.memset(ones_row, 1.0)
        ones1 = const.tile([1, 1], f16)
        nc.vector.memset(ones1, 1.0)
        dmy = const.tile([128, 512], f16)
        nc.vector.memset(dmy, 0.0)

        w_sb = wpool.tile([128, 6, NCC, HD], f16)
        wp_sb = wpool.tile([128, 2, C], f16)

        # ---------- chunk (QKV) machinery ----------
        xt_pend = {}
        sets = {}

        def dma_xt(c):
            b, qg = c
            t0 = b * T + qg * 512
            xt_t = xtp.tile([128, NCC, 512], f16, tag="xt", name=f"xt{b}{qg}")
            for g in range(4):
                nc.sync.dma_start(
                    xt_t[:, 4 * g:4 * g + 4, :], xv[:, 4 * g:4 * g + 4, t0:t0 + 512])
            xt_pend[c] = xt_t

        def alloc_set(b):
            qt = qkvp.tile([128, HPC, T], f16, tag="qt", name=f"qt{b}")
            kt = qkvp.tile([128, HPC, T], f16, tag="kt", name=f"kt{b}")
            v = qkvp.tile([128, T // 128, HPC * HD], f16, tag="v", name=f"v{b}")
            sets[b] = (qt, kt, v)

        def emit_qk_lump(c, j, kick):
            """16 W-stationary matmuls: one of q_h0/q_h1/k_h0/k_h1 for chunk c."""
            b, qg = c
            if b not in sets:
                alloc_set(b)
            qt, kt, v = sets[b]
            xt_t = xt_pend[c]
            qk_ps = ps.tile([128, 512], f32, tag="qk", bufs=1)
            for cc in range(NCC):
                nc.tensor.matmul(qk_ps, w_sb[:, j, cc, :], xt_t[:, cc, :],
                                 start=(cc == 0), stop=(cc == NCC - 1))
                if cc % 4 == 3:
                    kick()
            dst = (qt, qt, kt, kt)[j]
            nc.vector.tensor_copy(dst[:, j % 2, qg * 512:(qg + 1) * 512], qk_ps)

        def emit_v_lump(c, tb, kick, last=False):
            """16 x^T-stationary matmuls: V[t-block, 2*HD] for chunk c, direct
            [t, d] layout - no transposes."""
            b, qg = c
            if b not in sets:
                alloc_set(b)
            qt, kt, v = sets[b]
            xt_t = xt_pend[c]
            v_ps = ps.tile([128, 2 * HD], f32, tag="ov", bufs=2)
            for cc in range(NCC):
                nc.tensor.matmul(
                    v_ps, xt_t[:, cc, tb * 128:(tb + 1) * 128],
                    w_sb[:, 4:6, cc, :],
                    start=(cc == 0), stop=(cc == NCC - 1))
                if cc % 4 == 3:
                    kick()
            nc.vector.tensor_copy(v[:, qg * 4 + tb, :], v_ps)

        # ---------- attention pipeline (software-pipelined, depth 2) ----------
        PIPE = []

        def pipe_flush():
            kb, qs, st, u = PIPE.pop(0)
            pt, den = u["pt"], u["den"]
            nc.scalar.activation(
                pt[:, kb, qs:512], st[:, qs:512], Exp, scale=SCALE)
            if kb - 4 * u["qg"] >= 0:
                # causal zeroing of the upper triangle of the diagonal
                # 128x128 sub-block - on GPSIMD, off the PE/ACT/DVE hot paths
                nc.gpsimd.affine_select(
                    out=pt[:, kb, qs:qs + 128], in_=pt[:, kb, qs:qs + 128],
                    compare_op=IsGe, fill=0.0,
                    base=0, pattern=[[1, 128]], channel_multiplier=-1)
            if kb == 0:
                nc.vector.tensor_copy(den, pt[:, 0, :])
            else:
                nc.vector.tensor_add(
                    den[:, qs:512], den[:, qs:512], pt[:, kb, qs:512])
            nc.tensor.matmul(
                u["yt_ps"][:, qs:512], u["v_ap"][:, kb, :],
                pt[:, kb, qs:512],
                start=(kb == 0), stop=(kb == u["nkb"] - 1))

        def kick():
            if PIPE:
                pipe_flush()

        def pipe_push(e):
            PIPE.append(e)
            while len(PIPE) > 2:
                pipe_flush()

        def make_unit(b, qg, h):
            qt, kt, v = sets[b]
            return {
                "b": b, "qg": qg, "h": h, "nkb": 4 * qg + 4,
                "pt": ptp.tile([128, T // 128, 512], f16, tag="pt",
                               name=f"pt{b}{qg}{h}"),
                "den": dnp.tile([128, 512], f16, tag="den", name=f"dn{b}{qg}{h}"),
                "yt_ps": ps.tile([128, 512], f32, tag="yt", bufs=2,
                                 name=f"ytps{b}{qg}{h}"),
                "v_ap": v[:, :, h * HD:(h + 1) * HD],
            }

        def emit_block(u, kb):
            b, qg, h = u["b"], u["qg"], u["h"]
            qt, kt, v = sets[b]
            kk = kb - 4 * qg
            qs = max(0, kk) * 128
            st = ps.tile([128, 512], f32, tag="st", bufs=3)
            nc.tensor.matmul(
                st[:, qs:512], kt[:, h, kb * 128:(kb + 1) * 128],
                qt[:, h, qg * 512 + qs:(qg + 1) * 512],
                start=True, stop=True)
            pipe_push((kb, qs, st, u))

        # ---------- softmax epilogue (trail-1) ----------
        def emit_epiA(u):
            den_row = ps.tile([1, 512], f32, tag="ov", bufs=2,
                              name=f"dr{u['b']}{u['qg']}{u['h']}")
            nc.tensor.matmul(den_row, ones_col, u["den"], start=True, stop=True)
            rec = rp.tile([1, 512], f32, tag="rec")
            nc.vector.reciprocal_approx_fast(rec, den_row[0:1, :])
            rec16 = rp.tile([1, 512], f16, tag="rec16")
            nc.scalar.copy(rec16, rec)
            u["rec16"] = rec16

        def emit_epiB(u, yt):
            r_ps = ps.tile([128, 512], f32, tag="ov", bufs=2,
                           name=f"rps{u['b']}{u['qg']}{u['h']}")
            nc.tensor.matmul(r_ps, ones_row, u["rec16"], start=True, stop=True)
            r_sb = rp.tile([128, 512], f32, tag="rsb")
            nc.scalar.copy(r_sb, r_ps)
            nc.vector.tensor_mul(yt[:, u["h"], :], u["yt_ps"], r_sb)

        # ---------- output projection ----------
        osb_pend = {}

        def emit_proj_pair(b, qg, yt, tt, co, single_dma, act_frac=2):
            """act_frac=2: alternate evac DVE/ACT 1:1. act_frac=3: 1 in 3 on
            ACT (for the exp-saturated last step)."""
            if single_dma:
                # drain: the attention st/ov rings are idle by now - borrow
                # both (5 banks) so the final proj pairs are never gated on
                # PSUM-evacuation latency
                if (tt * 4 + co) % 2 == 0:
                    o_ps = ps.tile([128, 512], f32, tag="st", bufs=3,
                                   name=f"ops{b}{qg}{tt}{co}")
                else:
                    o_ps = ps.tile([128, 512], f32, tag="ov", bufs=2,
                                   name=f"ops{b}{qg}{tt}{co}")
            else:
                o_ps = ps.tile([128, 512], f32, tag="ov", bufs=2,
                               name=f"ops{b}{qg}{tt}{co}")
            for jh in range(HPC):
                nc.tensor.matmul(
                    o_ps, yt[:, jh, tt * 128:(tt + 1) * 128],
                    wp_sb[:, jh, co * 512:(co + 1) * 512],
                    start=(jh == 0), stop=(jh == HPC - 1))
            r0 = b * T + qg * 512 + tt * 128
            on_act = (tt * 4 + co) % act_frac == 0
            if single_dma:
                o_sb = op.tile([128, 512], f16, tag="osb1", bufs=4,
                               name=f"os{b}{qg}{tt}{co}")
                if not on_act:
                    nc.vector.tensor_copy(o_sb, o_ps)
                else:
                    nc.scalar.copy(o_sb, o_ps)
                nc.sync.dma_start(
                    out[r0:r0 + 128, co * 512:(co + 1) * 512], o_sb)
                return
            if co % 2 == 0:
                osb_pend[(b, qg)] = op.tile(
                    [128, 1024], f16, tag="osb", bufs=4, name=f"os{b}{qg}{tt}{co}")
            o_sb = osb_pend[(b, qg)]
            dst = o_sb[:, (co % 2) * 512:(co % 2 + 1) * 512]
            # alternate PSUM evacuation between DVE and ACT
            if not on_act:
                nc.vector.tensor_copy(dst, o_ps)
            else:
                nc.scalar.copy(dst, o_ps)
            if co % 2 == 1:
                c2 = co // 2
                nc.sync.dma_start(
                    out[r0:r0 + 128, c2 * 1024:(c2 + 1) * 1024], o_sb)

        # ---------- per-step interleaved emission ----------
        def emit_step(prev, idx, b, qg):
            nkb = 4 * qg + 4
            n2 = 2 * nkb
            ev = []
            seq_n = [0]

            def at(pos, fn):
                seq_n[0] += 1
                ev.append((pos, seq_n[0], fn))

            step_state = {"units": {}, "yt": None}

            def block_fn(h, kb):
                def f():
                    u = step_state["units"].get(h)
                    if u is None:
                        u = step_state["units"][h] = make_unit(b, qg, h)
                        if h == 0:
                            step_state["yt"] = ytp.tile(
                                [128, HPC, 512], f16, tag="yt",
                                name=f"yt{b}{qg}")
                    emit_block(u, kb)
                return f

            for h in range(2):
                for kb in range(nkb):
                    at(h * nkb + kb, block_fn(h, kb))

            if idx + 2 < len(SEQ):
                at(-1.0, (lambda c: lambda: dma_xt(c))(SEQ[idx + 2]))

            if idx + 1 < len(SEQ):
                nxt = SEQ[idx + 1]
                order = [("qk", 0), ("v", 0), ("qk", 2), ("v", 1),
                         ("qk", 1), ("v", 2), ("qk", 3), ("v", 3)]
                if idx == 14:
                    # defer most of chunk (3,3)'s QKV into the final step,
                    # which otherwise has no exp-independent PE filler and
                    # goes engine-bound (and lets HAM re-throttle the PE)
                    order = [("qk", 0), ("qk", 2)]
                for i, (kind, j) in enumerate(order):
                    if kind == "qk":
                        fn = (lambda jj: lambda: emit_qk_lump(nxt, jj, kick))(j)
                    else:
                        fn = (lambda tb: lambda: emit_v_lump(
                            nxt, tb, kick, last=(tb == 3)))(j)
                    at((i + 0.45) * n2 / len(order), fn)
            if idx == 15:
                cur = SEQ[15]
                deferred = [("v", 0, 0.7), ("v", 1, 3.4), ("qk", 1, 6.1),
                            ("v", 2, 9.0), ("v", 3, 12.0), ("qk", 3, 17.5)]
                for kind, j, pos in deferred:
                    if kind == "qk":
                        fn = (lambda jj: lambda: emit_qk_lump(cur, jj, kick))(j)
                    else:
                        fn = (lambda tb: lambda: emit_v_lump(
                            cur, tb, kick))(j)
                    at(pos, fn)

            if prev is not None:
                pu, pyt = prev["h1"], prev["yt"]
                pb, pqg = prev["bqg"]
                at(2.4, (lambda u: lambda: emit_epiA(u))(pu))
                at(4.4, (lambda u, y: lambda: emit_epiB(u, y))(pu, pyt))
                act_frac = 3 if idx == 15 else 2
                span = max(n2 - 6, 2)
                for i in range(16):
                    tt, co = divmod(i, 4)
                    at(5.5 + i * span / 16.0,
                       (lambda t_, c_, a_: lambda: emit_proj_pair(
                           pb, pqg, pyt, t_, c_, False, a_))(tt, co, act_frac))

            at(nkb + 2.4, lambda: emit_epiA(step_state["units"][0]))
            at(nkb + 4.4, lambda: emit_epiB(step_state["units"][0],
                                            step_state["yt"]))

            ev.sort(key=lambda e: (e[0], e[1]))
            for _, _, fn in ev:
                fn()
            return {"h1": step_state["units"][1], "yt": step_state["yt"],
                    "bqg": (b, qg), "prev": prev}

        # ---------- prologue: weights + first chunk ----------
        alloc_set(0)
        # warm-up matmuls: keep the PE busy >4us while the first DMAs land
        # so the HAM clock-gate reaches 8/8 before the real matmul stream
        wu_ps = ps.tile([1, 512], f32, tag="ov", bufs=2, name="wups")

        def warm(n=1):
            for _ in range(n):
                nc.tensor.matmul(wu_ps, ones_col, dmy, start=True, stop=True)

        warm(16)
        # startup DMAs interleaved in first-lump consumption order: 4-cc
        # groups of w_j0 / xt(0,0) / w_j2 so the first matmuls start after
        # ~0.6MB, not after the full 5MB
        xt00 = xtp.tile([128, NCC, 512], f16, tag="xt", name="xt00")
        xt_pend[(0, 0)] = xt00
        for g in range(4):
            nc.sync.dma_start(w_sb[:, 0, 4 * g:4 * g + 4, :],
                              wqkv_v[:, 0, 4 * g:4 * g + 4, :])
            nc.sync.dma_start(xt00[:, 4 * g:4 * g + 4, :],
                              xv[:, 4 * g:4 * g + 4, 0:512])
            nc.sync.dma_start(w_sb[:, 2, 4 * g:4 * g + 4, :],
                              wqkv_v[:, 2, 4 * g:4 * g + 4, :])
        nc.sync.dma_start(w_sb[:, 4], wqkv_v[:, 4])
        nc.sync.dma_start(w_sb[:, 5], wqkv_v[:, 5])
        nop = lambda: None
        # dummy-matmul kicks fill the DMA-starved stretches of the prologue
        # so the HAM clock-gate does not oscillate back to 4/8
        emit_qk_lump((0, 0), 0, warm)
        nc.sync.dma_start(w_sb[:, 1], wqkv_v[:, 1])
        nc.sync.dma_start(w_sb[:, 3], wqkv_v[:, 3])
        emit_qk_lump((0, 0), 2, warm)
        for tb in range(4):
            emit_v_lump((0, 0), tb, warm if tb < 2 else nop, last=(tb == 3))
        emit_qk_lump((0, 0), 1, nop)
        emit_qk_lump((0, 0), 3, nop)
        dma_xt((0, 1))
        nc.sync.dma_start(wp_sb, wproj_v)

        # ---------- main loop ----------
        prev = None
        for idx, (b, qg) in enumerate(SEQ):
            prev = emit_step(prev, idx, b, qg)

        # ---------- drain ----------
        # dummy matmuls keep the PE active through the serial softmax
        # epilogue chain so HAM stays at 8/8 for the final proj matmuls
        wu_d = ps.tile([1, 512], f32, tag="qk", bufs=1, name="wud")

        def warm_d(n):
            for _ in range(n):
                nc.tensor.matmul(wu_d, ones_col, dmy, start=True, stop=True)

        u1, yt_f = prev["h1"], prev["yt"]
        while PIPE:
            pipe_flush()
            warm_d(3)
        # h1's y^T evacuated UNNORMALIZED (no reciprocal dependency); 1/den
        # is folded into the proj evacuation below as a per-row scale, so the
        # final proj matmuls never wait on the softmax epilogue chain
        nc.vector.tensor_copy(yt_f[:, 1, :], u1["yt_ps"])
        warm_d(2)
        den_row = ps.tile([1, 512], f32, tag="ov", bufs=2, name="drD")
        nc.tensor.matmul(den_row, ones_col, u1["den"], start=True, stop=True)
        rec = rp.tile([1, 512], f32, tag="rec")
        nc.vector.reciprocal_approx_fast(rec, den_row[0:1, :])
        rec16d = rp.tile([1, 512], f16, tag="rec16")
        nc.scalar.copy(rec16d, rec)
        warm_d(2)
        # transpose 1/den to column form: [1,128].T @ [[1]] -> [128,1]
        rcol = rp.tile([128, 4], f32, tag="rcol", bufs=1)
        for tt in range(4):
            tp = ps.tile([128, 1], f32, tag="ov", bufs=2, name=f"tp{tt}")
            nc.tensor.matmul(tp, rec16d[0:1, tt * 128:(tt + 1) * 128], ones1,
                             start=True, stop=True)
            nc.vector.tensor_copy(rcol[:, tt:tt + 1], tp)
        Mult = mybir.AluOpType.mult
        Add = mybir.AluOpType.add
        for i in range(16):
            tt, co = divmod(i, 4)
            oA = ps.tile([128, 512], f32, tag="st", bufs=3, name=f"oA{tt}{co}")
            nc.tensor.matmul(oA, yt_f[:, 0, tt * 128:(tt + 1) * 128],
                             wp_sb[:, 0, co * 512:(co + 1) * 512],
                             start=True, stop=True)
            oB = ps.tile([128, 512], f32, tag="ov", bufs=2, name=f"oB{tt}{co}")
            nc.tensor.matmul(oB, yt_f[:, 1, tt * 128:(tt + 1) * 128],
                             wp_sb[:, 1, co * 512:(co + 1) * 512],
                             start=True, stop=True)
            o_sb = op.tile([128, 512], f16, tag="osb1", bufs=4,
                           name=f"od{tt}{co}")
            nc.scalar.copy(o_sb, oA)
            nc.vector.scalar_tensor_tensor(
                o_sb, oB, rcol[:, tt:tt + 1], o_sb, Mult, Add)
            r0 = 3 * T + 3 * 512 + tt * 128
            nc.sync.dma_start(
                out[r0:r0 + 128, co * 512:(co + 1) * 512], o_sb)

    nc.compile()
    return nc


def _get_nc():
    if "nc" not in _CACHE:
        _CACHE["nc"] = _build_nc()
    return _CACHE["nc"]


def _make_in_maps(x2d, Wqkv, Wproj):
    xT = np.ascontiguousarray(x2d.T).astype(np.float16)  # [C, B*T]
    in_maps = []
    for c in range(N_CORES):
        h0 = c * HPC
        cols = []
        for part in range(3):  # q, k, v blocks of Wqkv columns
            for h in range(HPC):
                j0 = part * C + (h0 + h) * HD
                cols.append(Wqkv[:, j0:j0 + HD])
        wq = np.ascontiguousarray(np.concatenate(cols, axis=1)).astype(np.float16)
        wp = np.ascontiguousarray(
            Wproj[h0 * HD:(h0 + HPC) * HD, :]).astype(np.float16)
        in_maps.append({"xt": xT, "wqkv": wq, "wproj": wp})
    return in_maps


def run_shards(in_maps, trace=False):
    from concourse.bass_utils import run_bass_kernel_spmd
    nc = _get_nc()
    last_err = None
    for _attempt in range(3):
        try:
            return run_bass_kernel_spmd(
                nc, in_maps, core_ids=list(range(N_CORES)), trace=trace)
        except Exception as e:  # transient NRT device errors — retry
            last_err = e
            if "UNAVAILABLE" not in str(e) and "UNRECOVERABLE" not in str(e):
                raise
    raise last_err


def kernel(x, Wqkv, Wproj):
    x = np.asarray(x, dtype=np.float32)
    Wqkv = np.asarray(Wqkv, dtype=np.float32)
    Wproj = np.asarray(Wproj, dtype=np.float32)
    x2d = np.ascontiguousarray(x.reshape(B * T, C))

    in_maps = _make_in_maps(x2d, Wqkv, Wproj)
    res = run_shards(in_maps)

    acc = res.results[0]["out"].astype(np.float32)
    for c in range(1, N_CORES):
        acc += res.results[c]["out"].astype(np.float32)
    return acc.reshape(B, T, C)


# revision 34
# speedup vs baseline: 1.0028x; 1.0002x over previous
"""Causal self-attention (B=4, T=2048, C=2048, H=16) on 8 trn2 NeuronCores.

Sharding: tensor-parallel over heads - 2 heads per core. Every core gets the
full (pre-transposed) activation xT, its 2 heads' slice of Wqkv columns and
Wproj rows, computes a full [B*T, C] partial output (fp16), and the host sums
the 8 partials (the "all-reduce after output projection" done host-side).

Per-core dataflow (all matmuls fp16 on PE), v2 schedule:
  * Q^T,K^T [d,t] via W-stationary matmuls; V [t,d] computed DIRECTLY via
    x^T-stationary matmuls (no PE transposes).
  * S = K^T-block.T @ Q^T chunks (PSUM f32) -> exp (ACT) -> causal zeroing of
    diagonal blocks on GPSIMD (affine_select, off the PE) -> P (fp16).
  * softmax denominator chased on DVE in fp16; partition-reduced by one
    ones-column matmul; reciprocal broadcast by one ones-row matmul.
  * The whole emission is a single interleaved stream: QKV matmuls of the
    NEXT chunk and output-projection matmuls of the PREVIOUS (b,qg) group are
    woven between attention S/PV blocks, so the in-order PE queue always has
    exp-independent work while ACT streams the exponentials.
  * PSUM budget (8 banks): S-blocks ring 3, QKV accum 1, PV accum ring 2,
    shared transient ring 2 (proj out / V accum / den row / recip bcast).
"""
import numpy as np

B, T, C = 4, 2048, 2048
H, HD = 16, 128
N_CORES = 8
HPC = H // N_CORES          # heads per core = 2
SCALE = float(1.0 / np.sqrt(HD))

_CACHE = {}


def _build_nc():
    import concourse.bass as bass
    from concourse import bacc
    import concourse.tile as tile
    import concourse.mybir as mybir
    from contextlib import ExitStack

    f32 = mybir.dt.float32
    f32r = mybir.dt.float32r
    f16 = mybir.dt.float16
    Exp = mybir.ActivationFunctionType.Exp
    IsGe = mybir.AluOpType.is_ge

    nc = bacc.Bacc("TRN2", target_bir_lowering=False, debug=False,
                   enable_asserts=True, num_devices=N_CORES)

    # Inputs (per-core shards prepared on host)
    xT = nc.dram_tensor("xt", [C, B * T], f16, kind="ExternalInput").ap()
    wqkv = nc.dram_tensor("wqkv", [C, 6 * HD], f16, kind="ExternalInput").ap()
    wproj = nc.dram_tensor("wproj", [HPC * HD, C], f16, kind="ExternalInput").ap()
    out = nc.dram_tensor("out", [B * T, C], f16, kind="ExternalOutput").ap()

    # DRAM views
    # j-major weight view: one DMA per qkv column-block j (j: q0,q1,k0,k1,v0,v1)
    wqkv_v = wqkv.rearrange("(cc p) (j d) -> p j cc d", p=128, d=HD)  # [128,6,16,128]
    wproj_v = wproj.rearrange("(jh p) c -> p jh c", p=128)            # [128,2,2048]
    xv = xT.rearrange("(cc p) t -> p cc t", p=128)                    # [128,16,8192]

    NCC = C // 128        # 16 contraction chunks
    SEQ = [(b, qg) for b in range(B) for qg in range(4)]

    with tile.TileContext(nc) as tc, ExitStack() as ctx:
        const = ctx.enter_context(tc.tile_pool(name="const", bufs=1))
        wpool = ctx.enter_context(tc.tile_pool(name="w", bufs=1))
        xtp = ctx.enter_context(tc.tile_pool(name="xt", bufs=2))
        qkvp = ctx.enter_context(tc.tile_pool(name="qkv", bufs=2))
        ptp = ctx.enter_context(tc.tile_pool(name="pt", bufs=2))
        dnp = ctx.enter_context(tc.tile_pool(name="dn", bufs=3))
        rp = ctx.enter_context(tc.tile_pool(name="r", bufs=2))
        ytp = ctx.enter_context(tc.tile_pool(name="yt", bufs=3))
        op = ctx.enter_context(tc.tile_pool(name="o", bufs=4))
        ps = ctx.enter_context(tc.tile_pool(name="ps", bufs=1, space="PSUM"))

        ones_col = const.tile([128, 1], f16)
        nc.vector.memset(ones_col, 1.0)
        ones_row = const.tile([1, 128], f16)
        nc.vector.memset(ones_row, 1.0)
        ones1 = const.tile([1, 1], f16)
        nc.vector.memset(ones1, 1.0)
        dmy = const.tile([128, 512], f16)
        nc.vector.memset(dmy, 0.0)

        w_sb = wpool.tile([128, 6, NCC, HD], f16)
        wp_sb = wpool.tile([128, 2, C], f16)

        # ---------- chunk (QKV) machinery ----------
        xt_pend = {}
        sets = {}

        def dma_xt(c):
            b, qg = c
            t0 = b * T + qg * 512
            xt_t = xtp.tile([128, NCC, 512], f16, tag="xt", name=f"xt{b}{qg}")
            for g in range(4):
                nc.sync.dma_start(
                    xt_t[:, 4 * g:4 * g + 4, :], xv[:, 4 * g:4 * g + 4, t0:t0 + 512])
            xt_pend[c] = xt_t

        def alloc_set(b):
            qt = qkvp.tile([128, HPC, T], f16, tag="qt", name=f"qt{b}")
            kt = qkvp.tile([128, HPC, T], f16, tag="kt", name=f"kt{b}")
            v = qkvp.tile([128, T // 128, HPC * HD], f16, tag="v", name=f"v{b}")
            sets[b] = (qt, kt, v)

        def emit_qk_lump(c, j, kick):
            """16 W-stationary matmuls: one of q_h0/q_h1/k_h0/k_h1 for chunk c."""
            b, qg = c
            if b not in sets:
                alloc_set(b)
            qt, kt, v = sets[b]
            xt_t = xt_pend[c]
            qk_ps = ps.tile([128, 512], f32, tag="qk", bufs=1)
            for cc in range(NCC):
                nc.tensor.matmul(qk_ps, w_sb[:, j, cc, :], xt_t[:, cc, :],
                                 start=(cc == 0), stop=(cc == NCC - 1))
                if cc % 4 == 3:
                    kick()
            dst = (qt, qt, kt, kt)[j]
            nc.vector.tensor_copy(dst[:, j % 2, qg * 512:(qg + 1) * 512], qk_ps)

        def emit_v_lump(c, tb, kick, last=False):
            """16 x^T-stationary matmuls: V[t-block, 2*HD] for chunk c, direct
            [t, d] layout - no transposes."""
            b, qg = c
            if b not in sets:
                alloc_set(b)
            qt, kt, v = sets[b]
            xt_t = xt_pend[c]
            v_ps = ps.tile([128, 2 * HD], f32, tag="ov", bufs=2)
            for cc in range(NCC):
                nc.tensor.matmul(
                    v_ps, xt_t[:, cc, tb * 128:(tb + 1) * 128],
                    w_sb[:, 4:6, cc, :],
                    start=(cc == 0), stop=(cc == NCC - 1))
                if cc % 4 == 3:
                    kick()
            nc.vector.tensor_copy(v[:, qg * 4 + tb, :], v_ps)

        # ---------- attention pipeline (software-pipelined, depth 2) ----------
        PIPE = []

        def pipe_flush():
            kb, qs, st, u = PIPE.pop(0)
            pt, den = u["pt"], u["den"]
            nc.scalar.activation(
                pt[:, kb, qs:512], st[:, qs:512], Exp, scale=SCALE)
            if kb - 4 * u["qg"] >= 0:
                # causal zeroing of the upper triangle of the diagonal
                # 128x128 sub-block - on GPSIMD, off the PE/ACT/DVE hot paths
                nc.gpsimd.affine_select(
                    out=pt[:, kb, qs:qs + 128], in_=pt[:, kb, qs:qs + 128],
                    compare_op=IsGe, fill=0.0,
                    base=0, pattern=[[1, 128]], channel_multiplier=-1)
            if kb == 0:
                nc.vector.tensor_copy(den, pt[:, 0, :])
            else:
                nc.vector.tensor_add(
                    den[:, qs:512], den[:, qs:512], pt[:, kb, qs:512])
            nc.tensor.matmul(
                u["yt_ps"][:, qs:512], u["v_ap"][:, kb, :],
                pt[:, kb, qs:512],
                start=(kb == 0), stop=(kb == u["nkb"] - 1))

        def kick():
            if PIPE:
                pipe_flush()

        def pipe_push(e):
            PIPE.append(e)
            while len(PIPE) > 2:
                pipe_flush()

        def make_unit(b, qg, h):
            qt, kt, v = sets[b]
            return {
                "b": b, "qg": qg, "h": h, "nkb": 4 * qg + 4,
                "pt": ptp.tile([128, T // 128, 512], f16, tag="pt",
                               name=f"pt{b}{qg}{h}"),
                "den": dnp.tile([128, 512], f16, tag="den", name=f"dn{b}{qg}{h}"),
                "yt_ps": ps.tile([128, 512], f32, tag="yt", bufs=2,
                                 name=f"ytps{b}{qg}{h}"),
                "v_ap": v[:, :, h * HD:(h + 1) * HD],
            }

        def emit_block(u, kb):
            b, qg, h = u["b"], u["qg"], u["h"]
            qt, kt, v = sets[b]
            kk = kb - 4 * qg
            qs = max(0, kk) * 128
            st = ps.tile([128, 512], f32, tag="st", bufs=3)
            nc.tensor.matmul(
                st[:, qs:512], kt[:, h, kb * 128:(kb + 1) * 128],
                qt[:, h, qg * 512 + qs:(qg + 1) * 512],
                start=True, stop=True)
            pipe_push((kb, qs, st, u))

        # ---------- softmax epilogue (trail-1) ----------
        def emit_epiA(u):
            den_row = ps.tile([1, 512], f32, tag="ov", bufs=2,
                              name=f"dr{u['b']}{u['qg']}{u['h']}")
            nc.tensor.matmul(den_row, ones_col, u["den"], start=True, stop=True)
            rec = rp.tile([1, 512], f32, tag="rec")
            nc.vector.reciprocal_approx_fast(rec, den_row[0:1, :])
            rec16 = rp.tile([1, 512], f16, tag="rec16")
            nc.scalar.copy(rec16, rec)
            u["rec16"] = rec16

        def emit_epiB(u, yt):
            r_ps = ps.tile([128, 512], f32, tag="ov", bufs=2,
                           name=f"rps{u['b']}{u['qg']}{u['h']}")
            nc.tensor.matmul(r_ps, ones_row, u["rec16"], start=True, stop=True)
            r_sb = rp.tile([128, 512], f32, tag="rsb")
            nc.scalar.copy(r_sb, r_ps)
            nc.vector.tensor_mul(yt[:, u["h"], :], u["yt_ps"], r_sb)

        # ---------- output projection ----------
        osb_pend = {}

        def emit_proj_pair(b, qg, yt, tt, co, single_dma, act_frac=2):
            """act_frac=2: alternate evac DVE/ACT 1:1. act_frac=3: 1 in 3 on
            ACT (for the exp-saturated last step)."""
            if single_dma:
                # drain: the attention st/ov rings are idle by now - borrow
                # both (5 banks) so the final proj pairs are never gated on
                # PSUM-evacuation latency
                if (tt * 4 + co) % 2 == 0:
                    o_ps = ps.tile([128, 512], f32, tag="st", bufs=3,
                                   name=f"ops{b}{qg}{tt}{co}")
                else:
                    o_ps = ps.tile([128, 512], f32, tag="ov", bufs=2,
                                   name=f"ops{b}{qg}{tt}{co}")
            else:
                o_ps = ps.tile([128, 512], f32, tag="ov", bufs=2,
                               name=f"ops{b}{qg}{tt}{co}")
            for jh in range(HPC):
                nc.tensor.matmul(
                    o_ps, yt[:, jh, tt * 128:(tt + 1) * 128],
                    wp_sb[:, jh, co * 512:(co + 1) * 512],
                    start=(jh == 0), stop=(jh == HPC - 1))
            r0 = b * T + qg * 512 + tt * 128
            on_act = (tt * 4 + co) % act_frac == 0
            if single_dma:
                o_sb = op.tile([128, 512], f16, tag="osb1", bufs=4,
                               name=f"os{b}{qg}{tt}{co}")
                if not on_act:
                    nc.vector.tensor_copy(o_sb, o_ps)
                else:
                    nc.scalar.copy(o_sb, o_ps)
                nc.sync.dma_start(
                    out[r0:r0 + 128, co * 512:(co + 1) * 512], o_sb)
                return
            if co % 2 == 0:
                osb_pend[(b, qg)] = op.tile(
                    [128, 1024], f16, tag="osb", bufs=4, name=f"os{b}{qg}{tt}{co}")
            o_sb = osb_pend[(b, qg)]
            dst = o_sb[:, (co % 2) * 512:(co % 2 + 1) * 512]
            # alternate PSUM evacuation between DVE and ACT
            if not on_act:
                nc.vector.tensor_copy(dst, o_ps)
            else:
                nc.scalar.copy(dst, o_ps)
            if co % 2 == 1:
                c2 = co // 2
                nc.sync.dma_start(
                    out[r0:r0 + 128, c2 * 1024:(c2 + 1) * 1024], o_sb)

        # ---------- per-step interleaved emission ----------
        def emit_step(prev, idx, b, qg):
            nkb = 4 * qg + 4
            n2 = 2 * nkb
            ev = []
            seq_n = [0]

            def at(pos, fn):
                seq_n[0] += 1
                ev.append((pos, seq_n[0], fn))

            step_state = {"units": {}, "yt": None}

            def block_fn(h, kb):
                def f():
                    u = step_state["units"].get(h)
                    if u is None:
                        u = step_state["units"][h] = make_unit(b, qg, h)
                        if h == 0:
                            step_state["yt"] = ytp.tile(
                                [128, HPC, 512], f16, tag="yt",
                                name=f"yt{b}{qg}")
                    emit_block(u, kb)
                return f

            for h in range(2):
                for kb in range(nkb):
                    at(h * nkb + kb, block_fn(h, kb))

            if idx + 2 < len(SEQ):
                at(-1.0, (lambda c: lambda: dma_xt(c))(SEQ[idx + 2]))

            if idx + 1 < len(SEQ):
                nxt = SEQ[idx + 1]
                order = [("qk", 0), ("v", 0), ("qk", 2), ("v", 1),
                         ("qk", 1), ("v", 2), ("qk", 3), ("v", 3)]
                if idx == 14:
                    # defer most of chunk (3,3)'s QKV into the final step,
                    # which otherwise has no exp-independent PE filler and
                    # goes engine-bound (and lets HAM re-throttle the PE)
                    order = [("qk", 0), ("qk", 2)]
                for i, (kind, j) in enumerate(order):
                    if kind == "qk":
                        fn = (lambda jj: lambda: emit_qk_lump(nxt, jj, kick))(j)
                    else:
                        fn = (lambda tb: lambda: emit_v_lump(
                            nxt, tb, kick, last=(tb == 3)))(j)
                    at((i + 0.45) * n2 / len(order), fn)
            if idx == 15:
                cur = SEQ[15]
                deferred = [("v", 0, 0.7), ("v", 1, 3.4), ("qk", 1, 6.1),
                            ("v", 2, 9.0), ("v", 3, 12.0), ("qk", 3, 17.5)]
                for kind, j, pos in deferred:
                    if kind == "qk":
                        fn = (lambda jj: lambda: emit_qk_lump(cur, jj, kick))(j)
                    else:
                        fn = (lambda tb: lambda: emit_v_lump(
                            cur, tb, kick))(j)
                    at(pos, fn)

            if prev is not None:
                pu, pyt = prev["h1"], prev["yt"]
                pb, pqg = prev["bqg"]
                at(2.4, (lambda u: lambda: emit_epiA(u))(pu))
                at(4.4, (lambda u, y: lambda: emit_epiB(u, y))(pu, pyt))
                act_frac = 3 if idx == 15 else 2
                if idx == 15:
                    # the last step's h0 half is filled by the deferred QKV
                    # lumps; concentrate proj filler in the exp-bound h1 half
                    pos0, span = 16.5, 14.5
                else:
                    pos0, span = 5.5, max(n2 - 6, 2)
                for i in range(16):
                    tt, co = divmod(i, 4)
                    at(pos0 + i * span / 16.0,
                       (lambda t_, c_, a_: lambda: emit_proj_pair(
                           pb, pqg, pyt, t_, c_, False, a_))(tt, co, act_frac))

            at(nkb + 2.4, lambda: emit_epiA(step_state["units"][0]))
            at(nkb + 4.4, lambda: emit_epiB(step_state["units"][0],
                                            step_state["yt"]))

            ev.sort(key=lambda e: (e[0], e[1]))
            for _, _, fn in ev:
                fn()
            return {"h1": step_state["units"][1], "yt": step_state["yt"],
                    "bqg": (b, qg), "prev": prev}

        # ---------- prologue: weights + first chunk ----------
        alloc_set(0)
        # warm-up matmuls: keep the PE busy >4us while the first DMAs land
        # so the HAM clock-gate reaches 8/8 before the real matmul stream
        wu_ps = ps.tile([1, 512], f32, tag="ov", bufs=2, name="wups")

        def warm(n=1):
            for _ in range(n):
                nc.tensor.matmul(wu_ps, ones_col, dmy, start=True, stop=True)

        warm(16)
        # startup DMAs interleaved in first-lump consumption order: 4-cc
        # groups of w_j0 / xt(0,0) / w_j2 so the first matmuls start after
        # ~0.6MB, not after the full 5MB
        xt00 = xtp.tile([128, NCC, 512], f16, tag="xt", name="xt00")
        xt_pend[(0, 0)] = xt00
        for g in range(4):
            nc.sync.dma_start(w_sb[:, 0, 4 * g:4 * g + 4, :],
                              wqkv_v[:, 0, 4 * g:4 * g + 4, :])
            nc.sync.dma_start(xt00[:, 4 * g:4 * g + 4, :],
                              xv[:, 4 * g:4 * g + 4, 0:512])
            nc.sync.dma_start(w_sb[:, 2, 4 * g:4 * g + 4, :],
                              wqkv_v[:, 2, 4 * g:4 * g + 4, :])
        nc.sync.dma_start(w_sb[:, 4], wqkv_v[:, 4])
        nc.sync.dma_start(w_sb[:, 5], wqkv_v[:, 5])
        nop = lambda: None
        # dummy-matmul kicks fill the DMA-starved stretches of the prologue
        # so the HAM clock-gate does not oscillate back to 4/8
        emit_qk_lump((0, 0), 0, warm)
        nc.sync.dma_start(w_sb[:, 1], wqkv_v[:, 1])
        nc.sync.dma_start(w_sb[:, 3], wqkv_v[:, 3])
        emit_qk_lump((0, 0), 2, warm)
        for tb in range(4):
            emit_v_lump((0, 0), tb, warm if tb < 2 else nop, last=(tb == 3))
        emit_qk_lump((0, 0), 1, nop)
        emit_qk_lump((0, 0), 3, nop)
        dma_xt((0, 1))
        nc.sync.dma_start(wp_sb, wproj_v)

        # ---------- main loop ----------
        prev = None
        for idx, (b, qg) in enumerate(SEQ):
            prev = emit_step(prev, idx, b, qg)

        # ---------- drain ----------
        # dummy matmuls keep the PE active through the serial softmax
        # epilogue chain so HAM stays at 8/8 for the final proj matmuls
        wu_d = ps.tile([1, 512], f32, tag="qk", bufs=1, name="wud")

        def warm_d(n):
            for _ in range(n):
                nc.tensor.matmul(wu_d, ones_col, dmy, start=True, stop=True)

        u1, yt_f = prev["h1"], prev["yt"]
        while PIPE:
            pipe_flush()
            warm_d(3)
        # h1's y^T evacuated UNNORMALIZED (no reciprocal dependency); 1/den
        # is folded into the proj evacuation below as a per-row scale, so the
        # final proj matmuls never wait on the softmax epilogue chain
        nc.vector.tensor_copy(yt_f[:, 1, :], u1["yt_ps"])
        warm_d(2)
        den_row = ps.tile([1, 512], f32, tag="ov", bufs=2, name="drD")
        nc.tensor.matmul(den_row, ones_col, u1["den"], start=True, stop=True)
        rec = rp.tile([1, 512], f32, tag="rec")
        nc.vector.reciprocal_approx_fast(rec, den_row[0:1, :])
        rec16d = rp.tile([1, 512], f16, tag="rec16")
        nc.scalar.copy(rec16d, rec)
        warm_d(2)
        # transpose 1/den to column form: [1,128].T @ [[1]] -> [128,1]
        rcol = rp.tile([128, 4], f32, tag="rcol", bufs=1)
        for tt in range(4):
            tp = ps.tile([128, 1], f32, tag="ov", bufs=2, name=f"tp{tt}")
            nc.tensor.matmul(tp, rec16d[0:1, tt * 128:(tt + 1) * 128], ones1,
                             start=True, stop=True)
            nc.vector.tensor_copy(rcol[:, tt:tt + 1], tp)
        Mult = mybir.AluOpType.mult
        Add = mybir.AluOpType.add
        for i in range(16):
            tt, co = divmod(i, 4)
            oA = ps.tile([128, 512], f32, tag="st", bufs=3, name=f"oA{tt}{co}")
            nc.tensor.matmul(oA, yt_f[:, 0, tt * 128:(tt + 1) * 128],
                             wp_sb[:, 0, co * 512:(co + 1) * 512],
                             start=True, stop=True)
            oB = ps.tile([128, 512], f32, tag="ov", bufs=2, name=f"oB{tt}{co}")
            nc.tensor.matmul(oB, yt_f[:, 1, tt * 128:(tt + 1) * 128],
                             wp_sb[:, 1, co * 512:(co + 1) * 512],
                             start=True, stop=True)
            o_sb = op.tile([128, 512], f16, tag="osb1", bufs=4,
                           name=f"od{tt}{co}")
            nc.scalar.copy(o_sb, oA)
            nc.vector.scalar_tensor_tensor(
                o_sb, oB, rcol[:, tt:tt + 1], o_sb, Mult, Add)
            r0 = 3 * T + 3 * 512 + tt * 128
            nc.sync.dma_start(
                out[r0:r0 + 128, co * 512:(co + 1) * 512], o_sb)

    nc.compile()
    return nc


def _get_nc():
    if "nc" not in _CACHE:
        _CACHE["nc"] = _build_nc()
    return _CACHE["nc"]


def _make_in_maps(x2d, Wqkv, Wproj):
    xT = np.ascontiguousarray(x2d.T).astype(np.float16)  # [C, B*T]
    in_maps = []
    for c in range(N_CORES):
        h0 = c * HPC
        cols = []
        for part in range(3):  # q, k, v blocks of Wqkv columns
            for h in range(HPC):
                j0 = part * C + (h0 + h) * HD
                cols.append(Wqkv[:, j0:j0 + HD])
        wq = np.ascontiguousarray(np.concatenate(cols, axis=1)).astype(np.float16)
        wp = np.ascontiguousarray(
            Wproj[h0 * HD:(h0 + HPC) * HD, :]).astype(np.float16)
        in_maps.append({"xt": xT, "wqkv": wq, "wproj": wp})
    return in_maps


def run_shards(in_maps, trace=False):
    from concourse.bass_utils import run_bass_kernel_spmd
    nc = _get_nc()
    last_err = None
    for _attempt in range(3):
        try:
            return run_bass_kernel_spmd(
                nc, in_maps, core_ids=list(range(N_CORES)), trace=trace)
        except Exception as e:  # transient NRT device errors — retry
            last_err = e
            if "UNAVAILABLE" not in str(e) and "UNRECOVERABLE" not in str(e):
                raise
    raise last_err


def kernel(x, Wqkv, Wproj):
    x = np.asarray(x, dtype=np.float32)
    Wqkv = np.asarray(Wqkv, dtype=np.float32)
    Wproj = np.asarray(Wproj, dtype=np.float32)
    x2d = np.ascontiguousarray(x.reshape(B * T, C))

    in_maps = _make_in_maps(x2d, Wqkv, Wproj)
    res = run_shards(in_maps)

    acc = res.results[0]["out"].astype(np.float32)
    for c in range(1, N_CORES):
        acc += res.results[c]["out"].astype(np.float32)
    return acc.reshape(B, T, C)


# revision 36
# speedup vs baseline: 1.0065x; 1.0038x over previous
"""Causal self-attention (B=4, T=2048, C=2048, H=16) on 8 trn2 NeuronCores.

Sharding: tensor-parallel over heads - 2 heads per core. Every core gets the
full (pre-transposed) activation xT, its 2 heads' slice of Wqkv columns and
Wproj rows, computes a full [B*T, C] partial output (fp16), and the host sums
the 8 partials (the "all-reduce after output projection" done host-side).

Per-core dataflow (all matmuls fp16 on PE), v2 schedule:
  * Q^T,K^T [d,t] via W-stationary matmuls; V [t,d] computed DIRECTLY via
    x^T-stationary matmuls (no PE transposes).
  * S = K^T-block.T @ Q^T chunks (PSUM f32) -> exp (ACT) -> causal zeroing of
    diagonal blocks on GPSIMD (affine_select, off the PE) -> P (fp16).
  * softmax denominator chased on DVE in fp16; partition-reduced by one
    ones-column matmul; reciprocal broadcast by one ones-row matmul.
  * The whole emission is a single interleaved stream: QKV matmuls of the
    NEXT chunk and output-projection matmuls of the PREVIOUS (b,qg) group are
    woven between attention S/PV blocks, so the in-order PE queue always has
    exp-independent work while ACT streams the exponentials.
  * PSUM budget (8 banks): S-blocks ring 3, QKV accum 1, PV accum ring 2,
    shared transient ring 2 (proj out / V accum / den row / recip bcast).
"""
import numpy as np

B, T, C = 4, 2048, 2048
H, HD = 16, 128
N_CORES = 8
HPC = H // N_CORES          # heads per core = 2
SCALE = float(1.0 / np.sqrt(HD))

_CACHE = {}


def _build_nc():
    import concourse.bass as bass
    from concourse import bacc
    import concourse.tile as tile
    import concourse.mybir as mybir
    from contextlib import ExitStack

    f32 = mybir.dt.float32
    f32r = mybir.dt.float32r
    f16 = mybir.dt.float16
    Exp = mybir.ActivationFunctionType.Exp
    IsGe = mybir.AluOpType.is_ge

    nc = bacc.Bacc("TRN2", target_bir_lowering=False, debug=False,
                   enable_asserts=True, num_devices=N_CORES)

    # Inputs (per-core shards prepared on host)
    xT = nc.dram_tensor("xt", [C, B * T], f16, kind="ExternalInput").ap()
    wqkv = nc.dram_tensor("wqkv", [C, 6 * HD], f16, kind="ExternalInput").ap()
    wproj = nc.dram_tensor("wproj", [HPC * HD, C], f16, kind="ExternalInput").ap()
    out = nc.dram_tensor("out", [B * T, C], f16, kind="ExternalOutput").ap()

    # DRAM views
    # j-major weight view: one DMA per qkv column-block j (j: q0,q1,k0,k1,v0,v1)
    wqkv_v = wqkv.rearrange("(cc p) (j d) -> p j cc d", p=128, d=HD)  # [128,6,16,128]
    wproj_v = wproj.rearrange("(jh p) c -> p jh c", p=128)            # [128,2,2048]
    xv = xT.rearrange("(cc p) t -> p cc t", p=128)                    # [128,16,8192]

    NCC = C // 128        # 16 contraction chunks
    SEQ = [(b, qg) for b in range(B) for qg in range(4)]

    with tile.TileContext(nc) as tc, ExitStack() as ctx:
        const = ctx.enter_context(tc.tile_pool(name="const", bufs=1))
        wpool = ctx.enter_context(tc.tile_pool(name="w", bufs=1))
        xtp = ctx.enter_context(tc.tile_pool(name="xt", bufs=2))
        qkvp = ctx.enter_context(tc.tile_pool(name="qkv", bufs=2))
        ptp = ctx.enter_context(tc.tile_pool(name="pt", bufs=2))
        dnp = ctx.enter_context(tc.tile_pool(name="dn", bufs=3))
        rp = ctx.enter_context(tc.tile_pool(name="r", bufs=2))
        ytp = ctx.enter_context(tc.tile_pool(name="yt", bufs=3))
        op = ctx.enter_context(tc.tile_pool(name="o", bufs=4))
        ps = ctx.enter_context(tc.tile_pool(name="ps", bufs=1, space="PSUM"))

        ones_col = const.tile([128, 1], f16)
        nc.vector.memset(ones_col, 1.0)
        ones_row = const.tile([1, 128], f16)
        nc.vector.memset(ones_row, 1.0)
        ones1 = const.tile([1, 1], f16)
        nc.vector.memset(ones1, 1.0)
        dmy = const.tile([128, 512], f16)
        nc.vector.memset(dmy, 0.0)

        w_sb = wpool.tile([128, 6, NCC, HD], f16)
        wp_sb = wpool.tile([128, 2, C], f16)

        # ---------- chunk (QKV) machinery ----------
        xt_pend = {}
        sets = {}

        def dma_xt(c):
            b, qg = c
            t0 = b * T + qg * 512
            xt_t = xtp.tile([128, NCC, 512], f16, tag="xt", name=f"xt{b}{qg}")
            for g in range(4):
                nc.sync.dma_start(
                    xt_t[:, 4 * g:4 * g + 4, :], xv[:, 4 * g:4 * g + 4, t0:t0 + 512])
            xt_pend[c] = xt_t

        def alloc_set(b):
            qt = qkvp.tile([128, HPC, T], f16, tag="qt", name=f"qt{b}")
            kt = qkvp.tile([128, HPC, T], f16, tag="kt", name=f"kt{b}")
            v = qkvp.tile([128, T // 128, HPC * HD], f16, tag="v", name=f"v{b}")
            sets[b] = (qt, kt, v)

        def emit_qk_lump(c, j, kick):
            """16 W-stationary matmuls: one of q_h0/q_h1/k_h0/k_h1 for chunk c."""
            b, qg = c
            if b not in sets:
                alloc_set(b)
            qt, kt, v = sets[b]
            xt_t = xt_pend[c]
            qk_ps = ps.tile([128, 512], f32, tag="qk", bufs=1)
            for cc in range(NCC):
                nc.tensor.matmul(qk_ps, w_sb[:, j, cc, :], xt_t[:, cc, :],
                                 start=(cc == 0), stop=(cc == NCC - 1))
                if cc % 4 == 3:
                    kick()
            dst = (qt, qt, kt, kt)[j]
            nc.vector.tensor_copy(dst[:, j % 2, qg * 512:(qg + 1) * 512], qk_ps)

        def emit_v_lump(c, tb, kick, last=False):
            """16 x^T-stationary matmuls: V[t-block, 2*HD] for chunk c, direct
            [t, d] layout - no transposes."""
            b, qg = c
            if b not in sets:
                alloc_set(b)
            qt, kt, v = sets[b]
            xt_t = xt_pend[c]
            v_ps = ps.tile([128, 2 * HD], f32, tag="ov", bufs=2)
            for cc in range(NCC):
                nc.tensor.matmul(
                    v_ps, xt_t[:, cc, tb * 128:(tb + 1) * 128],
                    w_sb[:, 4:6, cc, :],
                    start=(cc == 0), stop=(cc == NCC - 1))
                if cc % 4 == 3:
                    kick()
            nc.vector.tensor_copy(v[:, qg * 4 + tb, :], v_ps)

        # ---------- attention pipeline (software-pipelined, depth 2) ----------
        PIPE = []

        def pipe_flush():
            kb, qs, st, u = PIPE.pop(0)
            pt, den = u["pt"], u["den"]
            nc.scalar.activation(
                pt[:, kb, qs:512], st[:, qs:512], Exp, scale=SCALE)
            if kb - 4 * u["qg"] >= 0:
                # causal zeroing of the upper triangle of the diagonal
                # 128x128 sub-block - on GPSIMD, off the PE/ACT/DVE hot paths
                nc.gpsimd.affine_select(
                    out=pt[:, kb, qs:qs + 128], in_=pt[:, kb, qs:qs + 128],
                    compare_op=IsGe, fill=0.0,
                    base=0, pattern=[[1, 128]], channel_multiplier=-1)
            if kb == 0:
                nc.vector.tensor_copy(den, pt[:, 0, :])
            else:
                nc.vector.tensor_add(
                    den[:, qs:512], den[:, qs:512], pt[:, kb, qs:512])
            nc.tensor.matmul(
                u["yt_ps"][:, qs:512], u["v_ap"][:, kb, :],
                pt[:, kb, qs:512],
                start=(kb == 0), stop=(kb == u["nkb"] - 1))

        def kick():
            if PIPE:
                pipe_flush()

        def pipe_push(e):
            PIPE.append(e)
            while len(PIPE) > 2:
                pipe_flush()

        def make_unit(b, qg, h):
            qt, kt, v = sets[b]
            return {
                "b": b, "qg": qg, "h": h, "nkb": 4 * qg + 4,
                "pt": ptp.tile([128, T // 128, 512], f16, tag="pt",
                               name=f"pt{b}{qg}{h}"),
                "den": dnp.tile([128, 512], f16, tag="den", name=f"dn{b}{qg}{h}"),
                "yt_ps": ps.tile([128, 512], f32, tag="yt", bufs=2,
                                 name=f"ytps{b}{qg}{h}"),
                "v_ap": v[:, :, h * HD:(h + 1) * HD],
            }

        def emit_block(u, kb):
            b, qg, h = u["b"], u["qg"], u["h"]
            qt, kt, v = sets[b]
            kk = kb - 4 * qg
            qs = max(0, kk) * 128
            st = ps.tile([128, 512], f32, tag="st", bufs=3)
            nc.tensor.matmul(
                st[:, qs:512], kt[:, h, kb * 128:(kb + 1) * 128],
                qt[:, h, qg * 512 + qs:(qg + 1) * 512],
                start=True, stop=True)
            pipe_push((kb, qs, st, u))

        # ---------- softmax epilogue (trail-1) ----------
        def emit_epiA(u):
            den_row = ps.tile([1, 512], f32, tag="ov", bufs=2,
                              name=f"dr{u['b']}{u['qg']}{u['h']}")
            nc.tensor.matmul(den_row, ones_col, u["den"], start=True, stop=True)
            rec = rp.tile([1, 512], f32, tag="rec")
            nc.vector.reciprocal_approx_fast(rec, den_row[0:1, :])
            rec16 = rp.tile([1, 512], f16, tag="rec16")
            nc.scalar.copy(rec16, rec)
            u["rec16"] = rec16

        def emit_epiB(u, yt):
            r_ps = ps.tile([128, 512], f32, tag="ov", bufs=2,
                           name=f"rps{u['b']}{u['qg']}{u['h']}")
            nc.tensor.matmul(r_ps, ones_row, u["rec16"], start=True, stop=True)
            r_sb = rp.tile([128, 512], f32, tag="rsb")
            nc.scalar.copy(r_sb, r_ps)
            nc.vector.tensor_mul(yt[:, u["h"], :], u["yt_ps"], r_sb)

        # ---------- output projection ----------
        osb_pend = {}

        def emit_proj_pair(b, qg, yt, tt, co, single_dma, act_frac=2):
            """act_frac=2: alternate evac DVE/ACT 1:1. act_frac=3: 1 in 3 on
            ACT (for the exp-saturated last step)."""
            if single_dma:
                # drain: the attention st/ov rings are idle by now - borrow
                # both (5 banks) so the final proj pairs are never gated on
                # PSUM-evacuation latency
                if (tt * 4 + co) % 2 == 0:
                    o_ps = ps.tile([128, 512], f32, tag="st", bufs=3,
                                   name=f"ops{b}{qg}{tt}{co}")
                else:
                    o_ps = ps.tile([128, 512], f32, tag="ov", bufs=2,
                                   name=f"ops{b}{qg}{tt}{co}")
            else:
                o_ps = ps.tile([128, 512], f32, tag="ov", bufs=2,
                               name=f"ops{b}{qg}{tt}{co}")
            for jh in range(HPC):
                nc.tensor.matmul(
                    o_ps, yt[:, jh, tt * 128:(tt + 1) * 128],
                    wp_sb[:, jh, co * 512:(co + 1) * 512],
                    start=(jh == 0), stop=(jh == HPC - 1))
            r0 = b * T + qg * 512 + tt * 128
            on_act = (tt * 4 + co) % act_frac == 0
            if single_dma:
                o_sb = op.tile([128, 512], f16, tag="osb1", bufs=4,
                               name=f"os{b}{qg}{tt}{co}")
                if not on_act:
                    nc.vector.tensor_copy(o_sb, o_ps)
                else:
                    nc.scalar.copy(o_sb, o_ps)
                nc.sync.dma_start(
                    out[r0:r0 + 128, co * 512:(co + 1) * 512], o_sb)
                return
            if co % 2 == 0:
                osb_pend[(b, qg)] = op.tile(
                    [128, 1024], f16, tag="osb", bufs=4, name=f"os{b}{qg}{tt}{co}")
            o_sb = osb_pend[(b, qg)]
            dst = o_sb[:, (co % 2) * 512:(co % 2 + 1) * 512]
            # alternate PSUM evacuation between DVE and ACT
            if not on_act:
                nc.vector.tensor_copy(dst, o_ps)
            else:
                nc.scalar.copy(dst, o_ps)
            if co % 2 == 1:
                c2 = co // 2
                nc.sync.dma_start(
                    out[r0:r0 + 128, c2 * 1024:(c2 + 1) * 1024], o_sb)

        # ---------- per-step interleaved emission ----------
        def emit_step(prev, idx, b, qg):
            nkb = 4 * qg + 4
            n2 = 2 * nkb
            ev = []
            seq_n = [0]

            def at(pos, fn):
                seq_n[0] += 1
                ev.append((pos, seq_n[0], fn))

            step_state = {"units": {}, "yt": None}

            def block_fn(h, kb):
                def f():
                    u = step_state["units"].get(h)
                    if u is None:
                        u = step_state["units"][h] = make_unit(b, qg, h)
                        if h == 0:
                            step_state["yt"] = ytp.tile(
                                [128, HPC, 512], f16, tag="yt",
                                name=f"yt{b}{qg}")
                    emit_block(u, kb)
                return f

            for h in range(2):
                for kb in range(nkb):
                    at(h * nkb + kb, block_fn(h, kb))

            if idx + 2 < len(SEQ):
                at(-1.0, (lambda c: lambda: dma_xt(c))(SEQ[idx + 2]))

            if idx + 1 < len(SEQ):
                nxt = SEQ[idx + 1]
                order = [("qk", 0), ("v", 0), ("qk", 2), ("v", 1),
                         ("qk", 1), ("v", 2), ("qk", 3), ("v", 3)]
                if idx == 14:
                    # defer most of chunk (3,3)'s QKV into the final step,
                    # which otherwise has no exp-independent PE filler and
                    # goes engine-bound (and lets HAM re-throttle the PE)
                    order = [("qk", 0), ("qk", 2)]
                for i, (kind, j) in enumerate(order):
                    if kind == "qk":
                        fn = (lambda jj: lambda: emit_qk_lump(nxt, jj, kick))(j)
                    else:
                        fn = (lambda tb: lambda: emit_v_lump(
                            nxt, tb, kick, last=(tb == 3)))(j)
                    at((i + 0.45) * n2 / len(order), fn)
            if idx == 15:
                cur = SEQ[15]
                deferred = [("v", 0, 0.7), ("v", 1, 3.4), ("qk", 1, 6.1),
                            ("v", 2, 9.0), ("v", 3, 12.0), ("qk", 3, 17.5)]
                for kind, j, pos in deferred:
                    if kind == "qk":
                        fn = (lambda jj: lambda: emit_qk_lump(cur, jj, kick))(j)
                    else:
                        fn = (lambda tb: lambda: emit_v_lump(
                            cur, tb, kick))(j)
                    at(pos, fn)

            if prev is not None:
                pu, pyt = prev["h1"], prev["yt"]
                pb, pqg = prev["bqg"]
                at(2.4, (lambda u: lambda: emit_epiA(u))(pu))
                at(4.4, (lambda u, y: lambda: emit_epiB(u, y))(pu, pyt))
                act_frac = 3 if idx == 15 else 2
                if idx == 15:
                    # the last step's h0 half is filled by the deferred QKV
                    # lumps; concentrate proj filler in the exp-bound h1 half
                    pos0, span = 16.5, 14.5
                else:
                    pos0, span = 5.5, max(n2 - 6, 2)
                for i in range(16):
                    tt, co = divmod(i, 4)
                    at(pos0 + i * span / 16.0,
                       (lambda t_, c_, a_: lambda: emit_proj_pair(
                           pb, pqg, pyt, t_, c_, False, a_))(tt, co, act_frac))

            at(nkb + 2.4, lambda: emit_epiA(step_state["units"][0]))
            at(nkb + 4.4, lambda: emit_epiB(step_state["units"][0],
                                            step_state["yt"]))

            ev.sort(key=lambda e: (e[0], e[1]))
            for _, _, fn in ev:
                fn()
            return {"h1": step_state["units"][1], "yt": step_state["yt"],
                    "bqg": (b, qg), "prev": prev}

        # ---------- prologue: weights + first chunk ----------
        alloc_set(0)
        # warm-up matmuls: keep the PE busy >4us while the first DMAs land
        # so the HAM clock-gate reaches 8/8 before the real matmul stream
        wu_ps = ps.tile([1, 512], f32, tag="ov", bufs=2, name="wups")

        def warm(n=1):
            for _ in range(n):
                nc.tensor.matmul(wu_ps, ones_col, dmy, start=True, stop=True)

        warm(16)
        # startup DMAs interleaved in first-lump consumption order: 4-cc
        # groups of w_j0 / xt(0,0) / w_j2 so the first matmuls start after
        # ~0.6MB, not after the full 5MB
        xt00 = xtp.tile([128, NCC, 512], f16, tag="xt", name="xt00")
        xt_pend[(0, 0)] = xt00
        for g in range(4):
            nc.sync.dma_start(w_sb[:, 0, 4 * g:4 * g + 4, :],
                              wqkv_v[:, 0, 4 * g:4 * g + 4, :])
            nc.sync.dma_start(xt00[:, 4 * g:4 * g + 4, :],
                              xv[:, 4 * g:4 * g + 4, 0:512])
            nc.sync.dma_start(w_sb[:, 2, 4 * g:4 * g + 4, :],
                              wqkv_v[:, 2, 4 * g:4 * g + 4, :])
        nc.sync.dma_start(w_sb[:, 4], wqkv_v[:, 4])
        nc.sync.dma_start(w_sb[:, 5], wqkv_v[:, 5])
        nop = lambda: None
        # dummy-matmul kicks fill the DMA-starved stretches of the prologue
        # so the HAM clock-gate does not oscillate back to 4/8
        emit_qk_lump((0, 0), 0, warm)
        nc.sync.dma_start(w_sb[:, 1], wqkv_v[:, 1])
        nc.sync.dma_start(w_sb[:, 3], wqkv_v[:, 3])
        emit_qk_lump((0, 0), 2, warm)
        for tb in range(4):
            emit_v_lump((0, 0), tb, warm if tb < 2 else nop, last=(tb == 3))
        emit_qk_lump((0, 0), 1, nop)
        emit_qk_lump((0, 0), 3, nop)
        dma_xt((0, 1))
        nc.sync.dma_start(wp_sb, wproj_v)

        # ---------- main loop ----------
        prev = None
        for idx, (b, qg) in enumerate(SEQ):
            prev = emit_step(prev, idx, b, qg)

        # ---------- drain ----------
        # dummy matmuls keep the PE active through the serial softmax
        # epilogue chain so HAM stays at 8/8 for the final proj matmuls
        wu_d = ps.tile([1, 512], f32, tag="qk", bufs=1, name="wud")

        def warm_d(n):
            for _ in range(n):
                nc.tensor.matmul(wu_d, ones_col, dmy, start=True, stop=True)

        u1, yt_f = prev["h1"], prev["yt"]
        while PIPE:
            pipe_flush()
            warm_d(3)
        # h1's y^T evacuated UNNORMALIZED (no reciprocal dependency); 1/den
        # is folded into the proj evacuation below as a per-row scale, so the
        # final proj matmuls never wait on the softmax epilogue chain
        nc.vector.tensor_copy(yt_f[:, 1, :], u1["yt_ps"])
        warm_d(2)
        den_row = ps.tile([1, 512], f32, tag="ov", bufs=2, name="drD")
        nc.tensor.matmul(den_row, ones_col, u1["den"], start=True, stop=True)
        rec = rp.tile([1, 512], f32, tag="rec")
        nc.vector.reciprocal_approx_fast(rec, den_row[0:1, :])
        rec16d = rp.tile([1, 512], f16, tag="rec16")
        nc.scalar.copy(rec16d, rec)
        warm_d(2)
        # transpose 1/den to column form: [1,128].T @ [[1]] -> [128,1]
        rcol = rp.tile([128, 4], f32, tag="rcol", bufs=1)
        for tt in range(4):
            tp = ps.tile([128, 1], f32, tag="ov", bufs=2, name=f"tp{tt}")
            nc.tensor.matmul(tp, rec16d[0:1, tt * 128:(tt + 1) * 128], ones1,
                             start=True, stop=True)
            nc.vector.tensor_copy(rcol[:, tt:tt + 1], tp)
        Mult = mybir.AluOpType.mult
        Add = mybir.AluOpType.add
        # standard broadcast-normalize of h1 into a SEPARATE buffer, running
        # in parallel with the split-head pairs below (which read the
        # unnormalized yt_f and scale by 1/den during evacuation)
        r_ps = ps.tile([128, 512], f32, tag="ov", bufs=2, name="rpsd")
        nc.tensor.matmul(r_ps, ones_row, rec16d, start=True, stop=True)
        r_sb = rp.tile([128, 512], f32, tag="rsb")
        nc.scalar.copy(r_sb, r_ps)
        yt_n = ytp.tile([128, HPC, 512], f16, tag="yt", name="ytn")
        nc.vector.tensor_mul(yt_n[:, 1, :], u1["yt_ps"], r_sb)
        # pairs 0-5: split-head; proj matmuls never wait the chain above
        for i in range(6):
            tt, co = divmod(i, 4)
            oA = ps.tile([128, 512], f32, tag="st", bufs=3, name=f"oA{tt}{co}")
            nc.tensor.matmul(oA, yt_f[:, 0, tt * 128:(tt + 1) * 128],
                             wp_sb[:, 0, co * 512:(co + 1) * 512],
                             start=True, stop=True)
            oB = ps.tile([128, 512], f32, tag="ov", bufs=2, name=f"oB{tt}{co}")
            nc.tensor.matmul(oB, yt_f[:, 1, tt * 128:(tt + 1) * 128],
                             wp_sb[:, 1, co * 512:(co + 1) * 512],
                             start=True, stop=True)
            o_sb = op.tile([128, 512], f16, tag="osb1", bufs=4,
                           name=f"od{tt}{co}")
            nc.scalar.copy(o_sb, oA)
            nc.vector.scalar_tensor_tensor(
                o_sb, oB, rcol[:, tt:tt + 1], o_sb, Mult, Add)
            r0 = 3 * T + 3 * 512 + tt * 128
            nc.sync.dma_start(
                out[r0:r0 + 128, co * 512:(co + 1) * 512], o_sb)
        # pairs 6-15: combined accumulation (h0 from yt_f, h1 normalized)
        for i in range(6, 16):
            tt, co = divmod(i, 4)
            o_ps = ps.tile([128, 512], f32, tag="st", bufs=3,
                           name=f"oc{tt}{co}")
            nc.tensor.matmul(o_ps, yt_f[:, 0, tt * 128:(tt + 1) * 128],
                             wp_sb[:, 0, co * 512:(co + 1) * 512],
                             start=True, stop=False)
            nc.tensor.matmul(o_ps, yt_n[:, 1, tt * 128:(tt + 1) * 128],
                             wp_sb[:, 1, co * 512:(co + 1) * 512],
                             start=False, stop=True)
            o_sb = op.tile([128, 512], f16, tag="osb1", bufs=4,
                           name=f"oc{tt}{co}")
            if i % 2 == 0:
                nc.vector.tensor_copy(o_sb, o_ps)
            else:
                nc.scalar.copy(o_sb, o_ps)
            r0 = 3 * T + 3 * 512 + tt * 128
            nc.sync.dma_start(
                out[r0:r0 + 128, co * 512:(co + 1) * 512], o_sb)

    nc.compile()
    return nc


def _get_nc():
    if "nc" not in _CACHE:
        _CACHE["nc"] = _build_nc()
    return _CACHE["nc"]


def _make_in_maps(x2d, Wqkv, Wproj):
    xT = np.ascontiguousarray(x2d.T).astype(np.float16)  # [C, B*T]
    in_maps = []
    for c in range(N_CORES):
        h0 = c * HPC
        cols = []
        for part in range(3):  # q, k, v blocks of Wqkv columns
            for h in range(HPC):
                j0 = part * C + (h0 + h) * HD
                cols.append(Wqkv[:, j0:j0 + HD])
        wq = np.ascontiguousarray(np.concatenate(cols, axis=1)).astype(np.float16)
        wp = np.ascontiguousarray(
            Wproj[h0 * HD:(h0 + HPC) * HD, :]).astype(np.float16)
        in_maps.append({"xt": xT, "wqkv": wq, "wproj": wp})
    return in_maps


def run_shards(in_maps, trace=False):
    from concourse.bass_utils import run_bass_kernel_spmd
    nc = _get_nc()
    last_err = None
    for _attempt in range(3):
        try:
            return run_bass_kernel_spmd(
                nc, in_maps, core_ids=list(range(N_CORES)), trace=trace)
        except Exception as e:  # transient NRT device errors — retry
            last_err = e
            if "UNAVAILABLE" not in str(e) and "UNRECOVERABLE" not in str(e):
                raise
    raise last_err


def kernel(x, Wqkv, Wproj):
    x = np.asarray(x, dtype=np.float32)
    Wqkv = np.asarray(Wqkv, dtype=np.float32)
    Wproj = np.asarray(Wproj, dtype=np.float32)
    x2d = np.ascontiguousarray(x.reshape(B * T, C))

    in_maps = _make_in_maps(x2d, Wqkv, Wproj)
    res = run_shards(in_maps)

    acc = res.results[0]["out"].astype(np.float32)
    for c in range(1, N_CORES):
        acc += res.results[c]["out"].astype(np.float32)
    return acc.reshape(B, T, C)
